# revision 12
# baseline (speedup 1.0000x reference)
"""Trainium2 Bass kernel: EdgeModelConcat (GNN edge MLP).

reference math (per edge e):
    x   = concat([dest[e], src[e], u[batch[e]]])      # [192]
    h   = relu(x @ W1 + b1)                            # [256]
    out = h @ W2 + b2                                  # [64]
(edge_attr is an input but unused by the reference.)

Strategy
--------
Data-parallel over edges on 8 NeuronCores, all tensors bf16 on the wire:

* host passes x^T = [dest^T; src^T] as a [128, E/8] bf16 array per core, so
  layer-1 is h = W1[:128].T @ x^T with K=128, no on-device transposes.
* the u-term is folded away:  c[g] = u[g] @ W1[128:] + b1  is computed once
  on-device ([256, 512] f32 table).  `batch` is sorted, so per tile the
  bias column c[:, g] is piecewise constant; segment boundaries are baked
  into the instruction stream as static column ranges of the fused
  relu+bias evacuation ops.  Per-core segment structure differs -> one
  8-way tc.Switch on partition_id with per-core straight-line code.
* PSUM budget (8 banks): h0 pair-doubles [128,1024]x2 (relu pieces span
  tile boundaries in one op), h1 singles [128,512]x2, out singles
  [128,512]x2.
* layer-2 runs in quad groups (2 pairs): even tiles on PE col-group 0,
  odd tiles on col-group 1 (explicit tile_position) -> the two streams
  overlap in the array; chunk-major strip-adjacent order gets LDWEIGHTS
  elision (4 LDWs per 4 tiles).
* every PSUM->SBUF evacuation op (relu pieces + out bias-adds) is
  assigned to DVE or ACT by a greedy balancer over a measured cost model
  (DVE ~1.04ns/col + 170ns, ACT ~0.83ns/col + 300ns).
* outputs leave the device as bf16 [64, E] (pair-packed 2 tiles deep in
  128 partitions); the host unpacks and casts to f32.

Measured end-to-end rel-err ~3e-3 (gate 2e-2).
"""

import numpy as np

PROFILE = False            # set True (with NTFF hook installed) to measure
LAST_EXEC_NS = None        # exec time of slowest profiled core, ns
LAST_RESULTS = None

NCORES = 8
TILE = 512                 # edges per matmul tile (PSUM bank = 512 f32)
SLAB_TILES = 8             # tiles per DMA slab (4096 edges = 1MB bf16 in)

# evacuation cost model (ns) for the DVE/ACT greedy balancer
def _dve_cost(c):
    return 1.05 * c + 170.0

def _act_cost(c):
    return 0.84 * c + 250.0

_cache = {}


def _pair_segments(bk, ec, npairs):
    """bk: per-core sorted graph ids [ec] -> per pair, list of (sa, sb, g)
    with sa/sb column offsets in the pair-local [0, 1024) range."""
    out = []
    for p in range(npairs):
        c0 = p * 2 * TILE
        w = min(2 * TILE, ec - c0)
        vals = bk[c0 : c0 + w]
        bounds = np.flatnonzero(np.diff(vals)) + 1
        starts = np.concatenate([[0], bounds, [w]])
        out.append(
            [
                (int(starts[i]), int(starts[i + 1]), int(vals[starts[i]]))
                for i in range(len(starts) - 1)
            ]
        )
    return out


def _build(all_segs, ec, fx, fu, h, fo, b, out_w):
    from contextlib import ExitStack

    import concourse.bass as bass
    import concourse.mybir as mybir
    import concourse.tile as tile
    from concourse import bacc

    F32 = mybir.dt.float32
    BF16 = mybir.dt.bfloat16
    Relu = mybir.ActivationFunctionType.Relu
    Ident = mybir.ActivationFunctionType.Identity
    ADD = mybir.AluOpType.add
    MAX = mybir.AluOpType.max

    ntiles = (ec + TILE - 1) // TILE
    npairs = (ntiles + 1) // 2
    nquads = (npairs + 1) // 2
    nslabs = (ntiles + SLAB_TILES - 1) // SLAB_TILES
    slab = TILE * SLAB_TILES
    kin = 2 * fx            # 128: contraction dim of layer 1
    mh = h // 128           # 2: H chunks of 128
    assert kin == 128 and h == 256 and fo <= 64

    nc = bacc.Bacc("TRN2", target_bir_lowering=False, debug=False, num_devices=NCORES)
    # packed constants: cf (f32) = [uT | w1u | b1r | b2c] on 128 partitions,
    # cb (bf16) = [w1ds | w2c]
    cf_w = b + h + mh + 1
    cb_w = h + mh * fo
    xT = nc.declare_dram_parameter("xT", [kin, ec], BF16, isOutput=False)
    cf = nc.declare_dram_parameter("cf", [128, cf_w], F32, isOutput=False)
    cb = nc.declare_dram_parameter("cb", [128, cb_w], BF16, isOutput=False)
    outT = nc.declare_dram_parameter("outT", [128, out_w], BF16, isOutput=True)

    def tile_w(t):
        return min(TILE, ec - t * TILE)

    with tile.TileContext(nc) as tc, ExitStack() as ctx:
        pid = nc.partition_id()

        const = ctx.enter_context(tc.tile_pool(name="const", bufs=1))
        xp = ctx.enter_context(tc.tile_pool(name="xp", bufs=3))
        hp = ctx.enter_context(tc.tile_pool(name="hp", bufs=4))
        op = ctx.enter_context(tc.tile_pool(name="op", bufs=3))
        # 8 PSUM banks: h pair-doubles [128,1024]x3 (h0+h1 rotate through a
        # shared pool) + out quad-double [128,1024]x1
        ph = ctx.enter_context(tc.tile_pool(name="ph", bufs=3, space="PSUM"))
        po = ctx.enter_context(tc.tile_pool(name="po", bufs=1, space="PSUM"))

        cf_sb = const.tile([128, cf_w], F32)
        nc.sync.dma_start(cf_sb[:], cf[:])
        cb_sb = const.tile([128, cb_w], BF16)
        nc.sync.dma_start(cb_sb[:], cb[:])
        uT_sb = cf_sb[0:fu, 0:b]
        w1u_sb = cf_sb[0:fu, b : b + h]
        b1r_sb = cf_sb[:, b + h : b + h + mh]
        b2c_sb = cf_sb[:, b + h + mh : b + h + mh + 1]
        w1ds_sb = cb_sb[:, 0:h]
        w2c_sb = cb_sb[:, h : h + mh * fo]

        # c^T[m][:, g] = (u[g] @ W1[128:192] + b1)[128m : 128m+128], full fp32
        # (also serves as PE warm-up)
        cT_sb = const.tile([128, mh * b], F32)
        for m in range(mh):
            cps = po.tile([128, b], F32, tag="o", name="cps")
            nc.tensor.matmul(
                cps[:], w1u_sb[:, m * 128 : (m + 1) * 128], uT_sb[:],
                start=True, stop=True,
            )
            nc.scalar.activation(
                cT_sb[:, m * b : (m + 1) * b], cps[:], Ident,
                bias=b1r_sb[:, m : m + 1],
            )

        for core in tc.Switch(pid, NCORES):
            segs_per_pair = all_segs[core]
            xts = {}
            ots = {}
            hss = {}       # pair index -> hs sbuf tile [128, 4*TILE] bf16
            eng_load = [0.0, 0.0]   # DVE, ACT running cost

            def evac(cols, emit_dve, emit_act):
                # greedy: put this op on the engine that finishes it first
                d = eng_load[0] + _dve_cost(cols)
                a = eng_load[1] + _act_cost(cols)
                if d <= a:
                    eng_load[0] = d
                    emit_dve()
                else:
                    eng_load[1] = a
                    emit_act()

            def load_slab(s):
                if s in xts or s >= nslabs:
                    return
                c0 = s * slab
                ws = min(slab, ec - c0)
                xtn = xp.tile([kin, slab], BF16, tag="xt", name="xt")
                xts[s] = xtn
                # first slab lands in pair-sized chunks so the first L1
                # matmuls start as soon as their columns arrive (subtile
                # deps); later slabs arrive while the previous one computes
                nch = 4 if s == 0 else 1
                cw = slab // nch
                for i in range(nch):
                    lo = i * cw
                    hi = min(ws, lo + cw)
                    if lo < hi:
                        nc.sync.dma_start(
                            xtn[:, lo:hi], xT[:, c0 + lo : c0 + hi]
                        )
                ots[s] = op.tile([128, slab // 2], BF16, tag="ot", name="ot")

            def emit_l1(p):
                # tiles of this pair
                tps = [t for t in (2 * p, 2 * p + 1) if t < ntiles]
                for t in tps:
                    s, j = divmod(t, SLAB_TILES)
                    load_slab(s)
                    if j == 0:
                        load_slab(s + 1)
                hs = hp.tile([128, 4 * TILE], BF16, tag="hs", name="hs")
                hss[p] = hs
                # layer-1 matmuls, grouped by stationary chunk; h0/h1 each
                # land in a pair-double [128,1024] (two adjacent banks) so
                # relu pieces can straddle the tile boundary
                for m in range(mh):
                    hd = ph.tile([128, 2 * TILE], F32, tag="hd", name="hd")
                    for i, t in enumerate(tps):
                        s, j = divmod(t, SLAB_TILES)
                        xtt = xts[s]
                        a = j * TILE
                        w = tile_w(t)
                        nc.tensor.matmul(
                            hd[:, i * TILE : i * TILE + w],
                            w1ds_sb[:, m * 128 : (m + 1) * 128],
                            xtt[:, a : a + w], start=True, stop=True,
                        )
                    # fused bias+relu evacuation, greedy DVE/ACT, emitted
                    # right after this chunk's matmuls
                    for (sa, sb, g) in segs_per_pair[p]:
                        dst = hs[:, m * 2 * TILE + sa : m * 2 * TILE + sb]
                        src = hd[:, sa:sb]
                        cb_ = cT_sb[:, m * b + g : m * b + g + 1]
                        def dv(dst=dst, src=src, cb_=cb_):
                            nc.vector.tensor_scalar(
                                out=dst, in0=src, scalar1=cb_, scalar2=0.0,
                                op0=ADD, op1=MAX,
                            )
                        def ac(dst=dst, src=src, cb_=cb_):
                            nc.scalar.activation(dst, src, Relu, bias=cb_)
                        evac(sb - sa, dv, ac)

            oqs = {}   # quad index -> out psum quad-double [128, 2*TILE]

            def emit_l2_pair(p):
                # one pair's 4 L2 matmuls into its half of the quad-double;
                # even tile col-group 0, odd col-group 1 (streams overlap)
                q, pi = divmod(p, 2)
                if pi == 0:
                    oqs[q] = po.tile([128, 2 * TILE], F32, tag="o", name="o_q")
                o_q = oqs[q]
                hs = hss.pop(p)
                for m in range(mh):            # chunk-major
                    for r0 in (0, 64):         # strip-adjacent
                        t = 2 * p + (r0 // 64)
                        if t >= ntiles:
                            continue
                        w = tile_w(t)
                        rhs = hs[:, m * 2 * TILE + (r0 // 64) * TILE :][:, :w]
                        nc.tensor.matmul(
                            o_q[r0 : r0 + fo, pi * TILE : pi * TILE + w],
                            w2c_sb[:, m * fo : (m + 1) * fo],
                            rhs,
                            start=(m == 0), stop=(m == mh - 1),
                            tile_position=(0, r0),
                        )

            def emit_evac_quad(q):
                # bias-add evacuation into the out slab (one op per quad)
                o_q = oqs.pop(q)
                t0 = 4 * q
                s = t0 // SLAB_TILES
                cc = ((t0 % SLAB_TILES) // 2) * TILE
                ot = ots[s]
                nfull = min(t0 + 4, ntiles) - t0   # tiles in this quad
                # full-row span covers complete pairs; a trailing odd tile
                # only has rows 0:fo valid
                full_pairs_w = (nfull // 2) * TILE
                tail_w = (tile_w(min(t0 + nfull - 1, ntiles - 1))
                          if nfull % 2 else 0)
                def dveo(o_q=o_q, ot=ot, cc=cc, fw=full_pairs_w, tw=tail_w):
                    if fw:
                        nc.vector.tensor_scalar(
                            out=ot[:, cc : cc + fw], in0=o_q[:, :fw],
                            scalar1=b2c_sb[:], scalar2=None, op0=ADD,
                        )
                    if tw:
                        nc.vector.tensor_scalar(
                            out=ot[0:fo, cc + fw : cc + fw + tw],
                            in0=o_q[0:fo, fw : fw + tw],
                            scalar1=b2c_sb[0:fo, :], scalar2=None, op0=ADD,
                        )
                def acto(o_q=o_q, ot=ot, cc=cc, fw=full_pairs_w, tw=tail_w):
                    if fw:
                        nc.scalar.activation(
                            ot[:, cc : cc + fw], o_q[:, :fw], Ident,
                            bias=b2c_sb[:],
                        )
                    if tw:
                        nc.scalar.activation(
                            ot[0:fo, cc + fw : cc + fw + tw],
                            o_q[0:fo, fw : fw + tw], Ident,
                            bias=b2c_sb[0:fo, :],
                        )
                evac(full_pairs_w + tail_w, dveo, acto)
                # store the slab once its last quad is evacuated
                t_last = min(t0 + 3, ntiles - 1)
                if t_last == ntiles - 1 or t_last % SLAB_TILES == SLAB_TILES - 1:
                    nt = (t_last % SLAB_TILES) + 1
                    oc0 = (t_last // SLAB_TILES) * (slab // 2)
                    npr = nt // 2
                    if npr:
                        nc.sync.dma_start(
                            outT[:, oc0 : oc0 + npr * TILE],
                            ot[:, : npr * TILE],
                        )
                    if nt % 2:
                        cl = npr * TILE
                        wl = tile_w(ntiles - 1)
                        nc.sync.dma_start(
                            outT[0:fo, oc0 + cl : oc0 + cl + wl],
                            ot[0:fo, cl : cl + wl],
                        )

            L2_LAG = 2
            for p in range(npairs):
                emit_l1(p)
                pl = p - L2_LAG
                if pl >= 0:
                    emit_l2_pair(pl)
                    if pl % 2 == 1:
                        emit_evac_quad(pl // 2)
            for pl in range(max(npairs - L2_LAG, 0), npairs):
                emit_l2_pair(pl)
                if pl % 2 == 1:
                    emit_evac_quad(pl // 2)
            if (npairs - 1) % 2 == 0:
                emit_evac_quad((npairs - 1) // 2)
    nc.compile()
    return nc


def kernel(**inputs):
    global LAST_EXEC_NS, LAST_RESULTS

    import ml_dtypes

    npdt = np.dtype(ml_dtypes.bfloat16)

    src = np.asarray(inputs["src"], dtype=np.float32)
    dest = np.asarray(inputs["dest"], dtype=np.float32)
    u = np.asarray(inputs["u"], dtype=np.float32)
    batch = np.asarray(inputs["batch"])
    W1 = np.asarray(inputs["W1"], dtype=np.float32)
    b1 = np.asarray(inputs["b1"], dtype=np.float32)
    W2 = np.asarray(inputs["W2"], dtype=np.float32)
    b2 = np.asarray(inputs["b2"], dtype=np.float32)

    e, fx = src.shape
    b_, fu = u.shape
    h = W1.shape[1]
    fo = W2.shape[1]
    ec = (e + NCORES - 1) // NCORES
    ntiles = (ec + TILE - 1) // TILE
    npairs = (ntiles + 1) // 2

    # sorted edge order (identity when batch already sorted, as speced)
    bi = batch.astype(np.int64)
    if np.any(bi[1:] < bi[:-1]):
        perm = np.argsort(bi, kind="stable")
    else:
        perm = None

    # host-side marshalling ------------------------------------------------
    bs = bi if perm is None else bi[perm]
    all_segs = []
    in_maps = []

    # out column layout: tile t -> cols [S(t), S(t)+w) rows (t%2)*64 where
    # S(t) = (t//8)*8*TILE//2 + ((t%8)//2)*TILE
    def out_col(t):
        return (t // SLAB_TILES) * (SLAB_TILES // 2) * TILE + ((t % SLAB_TILES) // 2) * TILE

    wlast = ec - (ntiles - 1) * TILE
    out_w = max(out_col(ntiles - 1) + wlast, out_col(max(ntiles - 2, 0)) + TILE)

    mh = h // 128
    cf = np.zeros((128, b_ + h + mh + 1), dtype=np.float32)
    cf[:fu, :b_] = u.T
    cf[: W1.shape[0] - 2 * fx, b_ : b_ + h] = W1[2 * fx :]
    cf[:, b_ + h : b_ + h + mh] = b1.reshape(mh, 128).T
    cf[:, b_ + h + mh] = np.tile(b2, mh)
    cb = np.concatenate(
        [W1[: 2 * fx]]
        + [np.concatenate([W2[i * 128 : (i + 1) * 128] for i in range(mh)], axis=1)],
        axis=1,
    ).astype(npdt)
    cb = np.ascontiguousarray(cb)

    for k in range(NCORES):
        i0, i1 = k * ec, min((k + 1) * ec, e)
        n = i1 - i0
        if perm is None:
            d_k = dest[i0:i1]
            s_k = src[i0:i1]
        else:
            idx = perm[i0:i1]
            d_k = dest[idx]
            s_k = src[idx]
        xTk = np.empty((2 * fx, ec), dtype=npdt)
        xTk[:fx, :n] = d_k.T
        xTk[fx:, :n] = s_k.T
        if n < ec:
            xTk[:, n:] = 0
        bk = np.empty(ec, dtype=np.int64)
        bk[:n] = bs[i0:i1]
        if n < ec:
            bk[n:] = bk[n - 1]
        all_segs.append(_pair_segments(bk, ec, npairs))
        in_maps.append({"xT": xTk, "cf": cf, "cb": cb})

    # build / fetch compiled program --------------------------------------
    key = (e, fx, fu, h, fo, b_, hash(bs.tobytes()))
    nc = _cache.get(key)
    if nc is None:
        nc = _build(all_segs, ec, fx, fu, h, fo, b_, out_w)
        _cache.clear()
        _cache[key] = nc

    from concourse.bass_utils import run_bass_kernel_spmd

    res = run_bass_kernel_spmd(
        nc, in_maps, list(range(NCORES)), trace=bool(PROFILE)
    )
    LAST_EXEC_NS = res.exec_time_ns
    LAST_RESULTS = res

    # unpack ---------------------------------------------------------------
    out = np.empty((e, fo), dtype=np.float32)
    for k in range(NCORES):
        o = np.asarray(res.results[k]["outT"]).astype(np.float32)
        i0, i1 = k * ec, min((k + 1) * ec, e)
        n = i1 - i0
        ok = np.empty((ec, fo), dtype=np.float32)
        for t in range(ntiles):
            w = min(TILE, ec - t * TILE)
            c = out_col(t)
            r = (t % 2) * 64
            ok[t * TILE : t * TILE + w] = o[r : r + fo, c : c + w].T
        if perm is None:
            out[i0:i1] = ok[:n]
        else:
            out[perm[i0:i1]] = ok[:n]
    return out


if __name__ == "__main__":
    # small self-test with synthetic inputs (E scaled down)
    rng = np.random.default_rng(0)
    E, FX, FU, H, FO, B = 40960, 64, 64, 256, 64, 512
    src = rng.standard_normal((E, FX), dtype=np.float32)
    dest = rng.standard_normal((E, FX), dtype=np.float32)
    u = rng.standard_normal((B, FU), dtype=np.float32)
    batch = np.sort(rng.integers(0, B, E)).astype(np.int64)
    W1 = (rng.standard_normal((2 * FX + FU, H), dtype=np.float32) / np.sqrt(2 * FX + FU))
    b1 = np.zeros(H, np.float32)
    W2 = rng.standard_normal((H, FO), dtype=np.float32) / np.sqrt(H)
    b2 = np.zeros(FO, np.float32)
    got = kernel(src=src, dest=dest, edge_attr=src, u=u, batch=batch,
                 W1=W1, b1=b1, W2=W2, b2=b2)
    x = np.concatenate([dest, src, u[batch]], axis=1)
    hh = np.maximum(x @ W1 + b1, 0.0)
    want = hh @ W2 + b2
    rel = np.linalg.norm(got - want) / np.linalg.norm(want)
    print("rel err:", rel)
